# revision 1
# baseline (speedup 1.0000x reference)
"""Distributed Trainium2 kernel for LayerNorm -> biased multi-head attention -> out-proj.

Problem shapes (hardcoded):
  x        [4, 2048, 1024] f32
  attn_bias[16, 2048, 2048] f32
  ln_g/ln_b[1024] f32
  Wq       [1024, 1024] f32
  Wkv      [1024, 2048] f32
  Wout     [1024, 1024] f32
  out      [4, 2048, 1024] f32

Sharding: sequence-sharded over 8 cores; core r owns query rows
[r*256, (r+1)*256) of every batch. Per core: LN + q/k/v for its rows, then
k^T (inner-major) and v (token-major) are AllGathered in bf16 as 8
per-head-pair pieces so attention on pair i starts as soon as piece i has
arrived. Softmax = exp(sim)*exp(bias) (no max subtraction; values are
small); exp(bias) is transposed to kv-major via batched xbar DMA
transposes on the scalar ring. The softmax denominator comes from
co-executing ones-weight matmuls; attn@v packs the two heads of a pair
onto PE column halves via tile_position. PSUM accumulators are
zero-initialized with start=True zero-weight matmuls so interleaved
accumulation groups sharing a bank never clear each other's has_written
bits.
"""

import numpy as np

CORES = 8
B = 4
N = 2048
NLOC = N // CORES          # 256
ROWS = B * NLOC            # 1024 local query rows (row = b*NLOC + q)
DIM = 1024
H = 16
D = 64
PAIRS = H // 2             # head pairs
KC = N // 128              # 16 kv chunks of 128 tokens per batch
KP = 128 * DIM             # k^T part of one AG piece (elements)
VP = ROWS * 128            # v part of one AG piece
PIECE = KP + VP            # per-rank payload of one piece (bf16 elements)
SCALE = D ** -0.5
EPS = 1e-5

_CACHE = {}


def _build_nc():
    import contextlib
    import concourse.bass as bass
    import concourse.bacc as bacc
    import concourse.tile as tile
    import concourse.mybir as mybir
    from concourse import masks

    f32 = mybir.dt.float32
    bf16 = mybir.dt.bfloat16
    AF = mybir.ActivationFunctionType
    ALU = mybir.AluOpType

    nc = bacc.Bacc("TRN2", target_bir_lowering=False, debug=False,
                   num_devices=CORES)

    x_in = nc.dram_tensor("x", [ROWS, DIM], f32, kind="ExternalInput")
    bias_in = nc.dram_tensor("attn_bias", [H, NLOC, N], f32, kind="ExternalInput")
    ln_g = nc.dram_tensor("ln_g", [DIM], f32, kind="ExternalInput")
    ln_b = nc.dram_tensor("ln_b", [DIM], f32, kind="ExternalInput")
    wq_in = nc.dram_tensor("Wq", [DIM, DIM], f32, kind="ExternalInput")
    wkv_in = nc.dram_tensor("Wkv", [DIM, 2 * DIM], f32, kind="ExternalInput")
    wout_in = nc.dram_tensor("Wout", [DIM, DIM], f32, kind="ExternalInput")
    out_ext = nc.dram_tensor("out", [ROWS, DIM], f32, kind="ExternalOutput")

    with tile.TileContext(nc) as tc, contextlib.ExitStack() as top:
        # ------------------------------------------------------------------
        # DRAM scratch
        dram = top.enter_context(tc.tile_pool(name="dram", bufs=1, space="DRAM"))
        kv_loc = [dram.tile([PIECE], bf16, name=f"kvl{i}") for i in range(PAIRS)]
        kv_ful = [dram.tile([CORES * PIECE], bf16, name=f"kvf{i}",
                            addr_space="Shared") for i in range(PAIRS)]
        eb_dram = dram.tile([H, NLOC, N], bf16, name="eb_dram")
        biasT_dram = dram.tile([H, 2, 128, KC, 128], bf16, name="biasT_dram")

        # ------------------------------------------------------------------
        # Constants
        cpool = top.enter_context(tc.tile_pool(name="consts", bufs=1))
        identity = cpool.tile([128, 128], f32, name="identity")
        masks.make_identity(nc, identity[:])
        eps_t = cpool.tile([128, 1], f32, name="eps_t")
        nc.vector.memset(eps_t[:], EPS)
        # selector for denominator broadcast: sel[h, j] = 1 iff j//64 == h
        sel = cpool.tile([H, H * D], f32, name="sel")
        ones64 = cpool.tile([32, D], f32, name="ones64")
        nc.gpsimd.memset(sel[:], 0.0)
        nc.gpsimd.memset(ones64[:], 1.0)
        for h in range(H):
            nc.gpsimd.dma_start(sel[h:h + 1, h * D:(h + 1) * D], ones64[0:1, :])
        identity_bf = cpool.tile([128, 128], bf16, name="identity_bf")
        masks.make_identity(nc, identity_bf[:])
        zeros128 = cpool.tile([128, 128], bf16, name="zeros128")
        nc.vector.memset(zeros128[:], 0.0)
        ones1 = cpool.tile([128, 1], bf16, name="ones1")
        nc.vector.memset(ones1[:], 1.0)
        denom_asm = cpool.tile([H, ROWS], f32, name="denom_asm")

        # ------------------------------------------------------------------
        # Persistent pools (live until the end; LIFO with the top stack)
        qt_pool = top.enter_context(tc.tile_pool(name="qT", bufs=1))
        asm_pool = top.enter_context(tc.tile_pool(name="asm", bufs=1))
        asm = [asm_pool.tile([128, ROWS], bf16, name=f"asm{i}") for i in range(8)]
        bpool = top.enter_context(tc.tile_pool(name="biasin", bufs=2))
        epool = top.enter_context(tc.tile_pool(name="ebias", bufs=2))

        # ------------------------------------------------------------------
        # Weights: fp32 HWDGE loads + DVE casts (gpsimd cast-DMA is ~75 GB/s,
        # too slow). Emitted first so the sync ring drains them early.
        w_stack = contextlib.ExitStack()
        wq_pool = w_stack.enter_context(tc.tile_pool(name="wq", bufs=1))
        wkv_pool = w_stack.enter_context(tc.tile_pool(name="wkv", bufs=1))
        wstage_stack = contextlib.ExitStack()
        wstage = wstage_stack.enter_context(tc.tile_pool(name="wstage", bufs=2))
        wq_bf, wkv_bf = [], []
        for t in range(8):
            wf = wstage.tile([128, DIM], f32, name="ws")
            nc.gpsimd.dma_start(wf[:], wq_in[t * 128:(t + 1) * 128, :])
            wqt = wq_pool.tile([128, DIM], bf16, name=f"wq{t}")
            nc.vector.tensor_copy(wqt[:], wf[:])
            wq_bf.append(wqt)
        for t in range(8):
            wkt = wkv_pool.tile([128, 2 * DIM], bf16, name=f"wkv{t}")
            for half in range(2):
                wf = wstage.tile([128, DIM], f32, name="ws")
                nc.gpsimd.dma_start(
                    wf[:], wkv_in[t * 128:(t + 1) * 128,
                                  half * DIM:(half + 1) * DIM])
                nc.vector.tensor_copy(wkt[:, half * DIM:(half + 1) * DIM], wf[:])
            wkv_bf.append(wkt)

        # xnT pool opened before the LN pool so LN tiles can be freed first
        xnt_stack = contextlib.ExitStack()
        xnt_pool = xnt_stack.enter_context(tc.tile_pool(name="xnT", bufs=1))

        # ------------------------------------------------------------------
        # Phase 1: LayerNorm (rows on partitions) -> xn f32 in place
        ln_pool = contextlib.ExitStack()
        xpool = ln_pool.enter_context(tc.tile_pool(name="x", bufs=1))
        spool = ln_pool.enter_context(tc.tile_pool(name="stats", bufs=1))
        g_t = spool.tile([128, DIM], f32, name="g_t")
        b_t = spool.tile([128, DIM], f32, name="b_t")
        nc.gpsimd.dma_start(
            out=g_t[:],
            in_=bass.AP(tensor=ln_g.ap().tensor, offset=0, ap=[[0, 128], [1, DIM]]))
        nc.gpsimd.dma_start(
            out=b_t[:],
            in_=bass.AP(tensor=ln_b.ap().tensor, offset=0, ap=[[0, 128], [1, DIM]]))
        x_t = []
        for s in range(8):
            xt = xpool.tile([128, DIM], f32, name=f"x{s}")
            nc.gpsimd.dma_start(xt[:], x_in[s * 128:(s + 1) * 128, :])
            stats = spool.tile([128, 2, 6], f32, name=f"st{s}")
            mv = spool.tile([128, 2], f32, name=f"mv{s}")
            for g in range(2):
                nc.vector.bn_stats(stats[:, g], xt[:, g * 512:(g + 1) * 512])
            nc.vector.bn_aggr(mv[:], stats[:])
            # rstd = 1/sqrt(var + eps)
            nc.scalar.activation(mv[:, 1:2], mv[:, 1:2], AF.Sqrt,
                                 bias=eps_t[:, 0:1], scale=1.0)
            nc.vector.reciprocal(mv[:, 1:2], mv[:, 1:2])
            nc.vector.tensor_scalar(out=xt[:], in0=xt[:],
                                    scalar1=mv[:, 0:1], scalar2=mv[:, 1:2],
                                    op0=ALU.subtract, op1=ALU.mult)
            nc.vector.tensor_mul(xt[:], xt[:], g_t[:])
            nc.vector.tensor_add(xt[:], xt[:], b_t[:])
            x_t.append(xt)

        # ------------------------------------------------------------------
        # Phase 2: transpose xn -> xnT bf16 [dim-part, row-free]
        tr_stack = contextlib.ExitStack()
        tr_pool = tr_stack.enter_context(
            tc.tile_pool(name="trps", bufs=2, space="PSUM"))
        xnT = []
        for t in range(8):
            ps = tr_pool.tile([128, ROWS], f32, name="trp")
            for s in range(8):
                nc.tensor.transpose(ps[:, s * 128:(s + 1) * 128],
                                    x_t[s][:, t * 128:(t + 1) * 128],
                                    identity[:])
            xt_b = xnt_pool.tile([128, ROWS], bf16, name=f"xnT{t}")
            nc.vector.tensor_copy(xt_b[:], ps[:])
            xnT.append(xt_b)
        tr_stack.close()
        ln_pool.close()

        # ------------------------------------------------------------------
        # Phase 3: exp(bias) -> eb_dram -> batched xbar transposes ->
        # biasT_dram. The whole bias pipeline lives on the scalar ring so
        # the DMA-transpose <-> copy xbar-mode serialization never stalls
        # the sync ring; transposes are batched to limit mode switches.
        for h in range(H):
            for qh in range(2):
                for bh in range(2):
                    bt = bpool.tile([128, DIM], f32, name="bi")
                    nc.scalar.dma_start(
                        bt[:], bias_in[h, qh * 128:(qh + 1) * 128,
                                       bh * DIM:(bh + 1) * DIM])
                    et = epool.tile([128, DIM], bf16, name="eb")
                    nc.scalar.activation(et[:], bt[:], AF.Exp)
                    nc.sync.dma_start(
                        eb_dram[h, qh * 128:(qh + 1) * 128,
                                bh * DIM:(bh + 1) * DIM], et[:])

        # ------------------------------------------------------------------
        # Phase 4: QKV projections (bf16) + per-piece kv bounce + AllGathers
        qkv_psum_stack = contextlib.ExitStack()
        qkv_psum = qkv_psum_stack.enter_context(
            tc.tile_pool(name="qkvp", bufs=2, space="PSUM"))
        stage_stack = contextlib.ExitStack()
        stage_pool = stage_stack.enter_context(tc.tile_pool(name="kvstage", bufs=4))

        # v first (every piece needs all of v), then per pair: k piece +
        # AllGather; q last (only needed once attention starts).
        for s in range(8):
            ps = qkv_psum.tile([128, DIM], f32, name="qkvps")
            for ki in range(8):
                for nh in range(2):
                    nc.tensor.matmul(ps[:, nh * 512:(nh + 1) * 512],
                                     xnT[ki][:, s * 128:(s + 1) * 128],
                                     wkv_bf[ki][:, DIM + nh * 512:DIM + (nh + 1) * 512],
                                     start=(ki == 0), stop=(ki == 7))
            vst = stage_pool.tile([128, DIM], bf16, name="kvst")
            nc.vector.tensor_copy(vst[:], ps[:])
            for i in range(PAIRS):
                kvl = kv_loc[i][:]
                nc.sync.dma_start(
                    out=bass.AP(tensor=kvl.tensor,
                                offset=kvl.offset + KP + s * 128 * 128,
                                ap=[[128, 128], [1, 128]]),
                    in_=vst[:, i * 128:(i + 1) * 128])

        for i in range(PAIRS):
            ps = qkv_psum.tile([128, ROWS], f32, name="qkvps")
            for ki in range(8):
                for nh in range(2):
                    nc.tensor.matmul(ps[:, nh * 512:(nh + 1) * 512],
                                     wkv_bf[ki][:, i * 128:(i + 1) * 128],
                                     xnT[ki][:, nh * 512:(nh + 1) * 512],
                                     start=(ki == 0), stop=(ki == 7))
            kst = stage_pool.tile([128, ROWS], bf16, name="kvst")
            nc.vector.tensor_copy(kst[:], ps[:])
            kvl = kv_loc[i][:]
            nc.sync.dma_start(
                out=bass.AP(tensor=kvl.tensor, offset=kvl.offset,
                            ap=[[DIM, 128], [1, DIM]]),
                in_=kst[:])
            nc.gpsimd.collective_compute(
                "AllGather",
                mybir.AluOpType.bypass,
                replica_groups=[list(range(CORES))],
                ins=[kv_loc[i][:].opt()],
                outs=[kv_ful[i][:].opt()],
            )

        qT = []
        for mi in range(8):
            ps = qkv_psum.tile([128, ROWS], f32, name="qkvps")
            for ki in range(8):
                for nh in range(2):
                    nc.tensor.matmul(ps[:, nh * 512:(nh + 1) * 512],
                                     wq_bf[ki][:, mi * 128:(mi + 1) * 128],
                                     xnT[ki][:, nh * 512:(nh + 1) * 512],
                                     start=(ki == 0), stop=(ki == 7))
            qtile = qt_pool.tile([128, ROWS], bf16, name=f"qT{mi}")
            nc.vector.tensor_scalar_mul(qtile[:], ps[:], SCALE)
            qT.append(qtile)

        stage_stack.close()
        qkv_psum_stack.close()
        xnt_stack.close()
        wstage_stack.close()
        w_stack.close()

        # ------------------------------------------------------------------
        # Phase 4b: transpose exp(bias) on the PE (idle during the AllGather
        # flight). Zero xbar DMA transposes anywhere in the kernel, so the
        # global DMA-transpose<->copy serialization never engages.
        btr_stack = contextlib.ExitStack()
        ebld_pool = btr_stack.enter_context(tc.tile_pool(name="ebld", bufs=2))
        bst_pool = btr_stack.enter_context(tc.tile_pool(name="bst", bufs=4))
        btr_psum = btr_stack.enter_context(
            tc.tile_pool(name="btrp", bufs=2, space="PSUM"))
        bTd = biasT_dram[:]
        BTD_T = bTd.tensor
        for h in range(H):
            for qh in range(2):
                eb_t = ebld_pool.tile([128, N], bf16, name="ebld")
                nc.gpsimd.dma_start(eb_t[:], eb_dram[h, qh * 128:(qh + 1) * 128, :])
                for cg in range(2):
                    ps = btr_psum.tile([128, 8 * 128], bf16, name="btrp")
                    for cc in range(8):
                        c = cg * 8 + cc
                        nc.tensor.transpose(
                            ps[:, cc * 128:(cc + 1) * 128],
                            eb_t[:, c * 128:(c + 1) * 128],
                            identity_bf[:])
                    bst = bst_pool.tile([128, 8 * 128], bf16, name="bst")
                    nc.vector.tensor_copy(bst[:], ps[:])
                    # store as biasT2[h][qh][p][c][q']
                    nc.scalar.dma_start(
                        out=bass.AP(
                            tensor=BTD_T,
                            offset=(bTd.offset + (h * 2 + qh) * 128 * KC * 128
                                    + cg * 8 * 128),
                            ap=[[KC * 128, 128], [128, 8], [1, 128]]),
                        in_=bst[:])
        btr_stack.close()

        # ------------------------------------------------------------------
        # Phase 5: attention over head pairs (software-pipelined over chunks)

        attn_stack = contextlib.ExitStack()
        kt_pool = attn_stack.enter_context(tc.tile_pool(name="kT", bufs=3))
        vt_pool = attn_stack.enter_context(tc.tile_pool(name="vt", bufs=2))
        ebt_pool = attn_stack.enter_context(tc.tile_pool(name="ebT", bufs=6))
        ae_pool = attn_stack.enter_context(tc.tile_pool(name="ae", bufs=6))
        den_pool = attn_stack.enter_context(tc.tile_pool(name="den", bufs=2))
        sim_psum = attn_stack.enter_context(
            tc.tile_pool(name="simp", bufs=2, space="PSUM"))
        out_psum = attn_stack.enter_context(
            tc.tile_pool(name="outp", bufs=2, space="PSUM"))
        dn_psum = attn_stack.enter_context(
            tc.tile_pool(name="dnp", bufs=2, space="PSUM"))

        for i in range(PAIRS):
            kvf = kv_ful[i][:]
            KVF_T = kvf.tensor
            # k^T for the head pair: [128 (2 heads x 64 d), B*N] bf16
            kt = kt_pool.tile([128, B * N], bf16, name="kt")
            ktd = kt[:]
            for b in range(B):
                nc.sync.dma_start(
                    out=bass.AP(tensor=ktd.tensor, offset=ktd.offset + b * N,
                                ap=[ktd.ap[0], [NLOC, CORES], [1, NLOC]]),
                    in_=bass.AP(tensor=KVF_T,
                                offset=kvf.offset + b * NLOC,
                                ap=[[DIM, 128], [PIECE, CORES], [1, NLOC]]))
            # v for both heads of the pair: [128 tok, (b,c) x 128 inner]
            vt = vt_pool.tile([128, B * KC * 128], bf16, name="vt")
            vb = vt[:]
            for b in range(B):
                for c2 in range(2):
                    nc.sync.dma_start(
                        out=bass.AP(tensor=vb.tensor,
                                    offset=(vb.offset + (b * KC + c2) * 128),
                                    ap=[vb.ap[0], [256, CORES], [1, 128]]),
                        in_=bass.AP(tensor=KVF_T,
                                    offset=(kvf.offset + KP
                                            + (b * NLOC + c2 * 128) * 128),
                                    ap=[[128, 128], [PIECE, CORES], [1, 128]]))
            # exp(bias)^T for both heads: [128 kv, KC*NLOC] from biasT_dram
            ebt = [None, None]
            for parity in range(2):
                et = ebt_pool.tile([128, KC * NLOC], bf16, name="ebt")
                ebt[parity] = et
                h = 2 * i + parity
                etd = et[:]
                for qh in range(2):
                    nc.sync.dma_start(
                        out=bass.AP(tensor=etd.tensor,
                                    offset=etd.offset + qh * 128,
                                    ap=[etd.ap[0], [NLOC, KC], [1, 128]]),
                        in_=bass.AP(
                            tensor=BTD_T,
                            offset=bTd.offset + (h * 2 + qh) * 128 * KC * 128,
                            ap=[[KC * 128, 128], [128, KC], [1, 128]]))

            po, dn = {}, {}
            for bp in range(2):
                p_t = out_psum.tile([128, 512], f32, name="po")
                po[bp] = p_t
                nc.tensor.matmul(p_t[:, :], zeros128[:], qT[i][:, 0:512],
                                 start=True, stop=False, skip_group_check=True)
                d_t = dn_psum.tile([33, 512], f32, name="dn")
                dn[bp] = d_t
                nc.tensor.matmul(d_t[:, :], zeros128[:, 0:33], qT[i][:, 0:512],
                                 start=True, stop=False, skip_group_check=True)

            ae_ring = {}
            for c in range(KC + 1):
                if c < KC:
                    pss = {}
                    for parity in range(2):
                        pss[parity] = sim_psum.tile([128, B * NLOC], f32,
                                                    name="simps")
                    # interleave parities so consecutive LDWEIGHTS hit
                    # different PE row-groups and overlap the matmuls
                    for b in range(B):
                        for parity in range(2):
                            nc.tensor.matmul(
                                pss[parity][:, b * NLOC:(b + 1) * NLOC],
                                kt[parity * 64:parity * 64 + 64,
                                   b * N + c * 128:b * N + (c + 1) * 128],
                                qT[i][parity * 64:parity * 64 + 64,
                                      b * NLOC:(b + 1) * NLOC],
                                start=True, stop=True,
                                tile_position=(parity * 64, 0))
                    for parity in range(2):
                        ae = ae_pool.tile([128, B * NLOC], bf16, name="ae")
                        nc.scalar.activation(ae[:], pss[parity][:], AF.Exp)
                        ebs = ebt[parity][:, c * NLOC:(c + 1) * NLOC]
                        bcast = bass.AP(tensor=ebs.tensor, offset=ebs.offset,
                                        ap=[ebs.ap[0], [0, B], [1, NLOC]])
                        ae3 = ae[:].rearrange("p (b q) -> p b q", b=B)
                        nc.vector.tensor_tensor(out=ae3, in0=ae3, in1=bcast,
                                                op=ALU.mult)
                        ae_ring[c, parity] = ae
                if c >= 1:
                    cp = c - 1
                    ae_e = ae_ring.pop((cp, 0))
                    ae_o = ae_ring.pop((cp, 1))
                    for b in range(B):
                        blk = (b * KC + cp) * 128
                        # attn@v: head-even -> PE cols 0-63, head-odd ->
                        # cols 64-127; the two matmuls co-execute
                        nc.tensor.matmul(
                            po[b // 2][0:64, (b % 2) * NLOC:((b % 2) + 1) * NLOC],
                            vt[:, blk:blk + 64],
                            ae_e[:, b * NLOC:(b + 1) * NLOC],
                            start=False, stop=(cp == KC - 1),
                            tile_position=(0, 0), skip_group_check=True)
                        nc.tensor.matmul(
                            po[b // 2][64:128, (b % 2) * NLOC:((b % 2) + 1) * NLOC],
                            vt[:, blk + 64:blk + 128],
                            ae_o[:, b * NLOC:(b + 1) * NLOC],
                            start=False, stop=(cp == KC - 1),
                            tile_position=(0, 64), skip_group_check=True)
                    for bp in range(2):
                        # denominators: batched ones-weight matmuls (N=512)
                        nc.tensor.matmul(
                            dn[bp][0:1, :], ones1[:],
                            ae_e[:, bp * 512:(bp + 1) * 512],
                            start=False, stop=(cp == KC - 1),
                            tile_position=(0, 0), skip_group_check=True)
                        nc.tensor.matmul(
                            dn[bp][32:33, :], ones1[:],
                            ae_o[:, bp * 512:(bp + 1) * 512],
                            start=False, stop=(cp == KC - 1),
                            tile_position=(0, 32), skip_group_check=True)

            # evacuate pair outputs + denominators
            den_row = den_pool.tile([33, ROWS], f32, name="den")
            for bp in range(2):
                nc.vector.tensor_copy(
                    asm[i][:, bp * 512:(bp + 1) * 512], po[bp][:, :])
                nc.vector.tensor_copy(
                    den_row[0:1, bp * 512:(bp + 1) * 512], dn[bp][0:1, :])
                nc.vector.tensor_copy(
                    den_row[32:33, bp * 512:(bp + 1) * 512], dn[bp][32:33, :])
            nc.scalar.dma_start(denom_asm[2 * i:2 * i + 1, :], den_row[0:1, :])
            nc.scalar.dma_start(denom_asm[2 * i + 1:2 * i + 2, :],
                                den_row[32:33, :])

        attn_stack.close()

        # ------------------------------------------------------------------
        # Phase 6: normalize by softmax denominator
        fin_stack = contextlib.ExitStack()
        rb_psum = fin_stack.enter_context(
            tc.tile_pool(name="rbp", bufs=2, space="PSUM"))
        rb_pool = fin_stack.enter_context(tc.tile_pool(name="rbs", bufs=2))
        recip = cpool.tile([H, ROWS], f32, name="recip")
        nc.vector.reciprocal(recip[:], denom_asm[:])
        for i in range(8):
            ps = rb_psum.tile([128, ROWS], f32, name="rbp")
            for nh in range(2):
                nc.tensor.matmul(ps[:, nh * 512:(nh + 1) * 512],
                                 sel[:, i * 128:(i + 1) * 128],
                                 recip[:, nh * 512:(nh + 1) * 512],
                                 start=True, stop=True)
            rbs = rb_pool.tile([128, ROWS], f32, name="rbs")
            nc.vector.tensor_copy(rbs[:], ps[:])
            nc.vector.tensor_mul(asm[i][:], asm[i][:], rbs[:])

        # ------------------------------------------------------------------
        # Phase 7: output projection
        wo_pool = fin_stack.enter_context(tc.tile_pool(name="wo", bufs=1))
        wos_pool = fin_stack.enter_context(tc.tile_pool(name="wos", bufs=2))
        wout_bf = []
        for t in range(8):
            wf = wos_pool.tile([128, DIM], f32, name="wsf")
            nc.sync.dma_start(wf[:], wout_in[t * 128:(t + 1) * 128, :])
            wot = wo_pool.tile([128, DIM], bf16, name=f"wo{t}")
            nc.vector.tensor_copy(wot[:], wf[:])
            wout_bf.append(wot)
        f_psum = fin_stack.enter_context(
            tc.tile_pool(name="fp", bufs=2, space="PSUM"))
        o_pool = fin_stack.enter_context(tc.tile_pool(name="osb", bufs=3))
        for mi in range(8):
            ps = f_psum.tile([128, DIM], f32, name="fp")
            for ki in range(8):
                for nh in range(2):
                    nc.tensor.matmul(ps[:, nh * 512:(nh + 1) * 512],
                                     asm[ki][:, mi * 128:(mi + 1) * 128],
                                     wout_bf[ki][:, nh * 512:(nh + 1) * 512],
                                     start=(ki == 0), stop=(ki == 7))
            ot = o_pool.tile([128, DIM], f32, name="ot")
            nc.vector.tensor_copy(ot[:], ps[:])
            nc.sync.dma_start(out_ext[mi * 128:(mi + 1) * 128, :], ot[:])

        fin_stack.close()

    nc.finalize()
    return nc


def _get_nc():
    if "nc" not in _CACHE:
        _CACHE["nc"] = _build_nc()
    return _CACHE["nc"]


def kernel(x, attn_bias, ln_g, ln_b, Wq, Wkv, Wout):
    from concourse import bass_utils

    nc = _get_nc()
    x = np.asarray(x, dtype=np.float32)
    attn_bias = np.asarray(attn_bias, dtype=np.float32)
    in_maps = []
    for r in range(CORES):
        in_maps.append({
            "x": np.ascontiguousarray(
                x[:, r * NLOC:(r + 1) * NLOC, :]).reshape(ROWS, DIM),
            "attn_bias": np.ascontiguousarray(
                attn_bias[:, r * NLOC:(r + 1) * NLOC, :]),
            "ln_g": np.asarray(ln_g, dtype=np.float32),
            "ln_b": np.asarray(ln_b, dtype=np.float32),
            "Wq": np.asarray(Wq, dtype=np.float32),
            "Wkv": np.asarray(Wkv, dtype=np.float32),
            "Wout": np.asarray(Wout, dtype=np.float32),
        })
    res = bass_utils.run_bass_kernel_spmd(nc, in_maps, core_ids=list(range(CORES)))
    out = np.empty((B, N, DIM), dtype=np.float32)
    for r in range(CORES):
        out[:, r * NLOC:(r + 1) * NLOC, :] = \
            res.results[r]["out"].reshape(B, NLOC, DIM)
    return out



# revision 4
# speedup vs baseline: 1.4384x; 1.4384x over previous
"""Distributed Trainium2 kernel for LayerNorm -> biased multi-head attention -> out-proj.

Problem shapes (hardcoded):
  x        [4, 2048, 1024] f32
  attn_bias[16, 2048, 2048] f32
  ln_g/ln_b[1024] f32
  Wq       [1024, 1024] f32
  Wkv      [1024, 2048] f32
  Wout     [1024, 1024] f32
  out      [4, 2048, 1024] f32

Sharding: sequence-sharded over 8 cores; core r owns query rows
[r*256, (r+1)*256) of every batch. Host-side prep (layout only): weights are
cast to bf16; attn_bias is sliced per core, transposed to kv-major
[H, 128, KC*256] bf16 (kv = c*128 + p) so the device consumes it directly
with contiguous 8KB DMA lines and a single exp() per head.

Per core: LN + q/k/v for its rows; k^T (inner-major) and v (token-major) are
AllGathered in bf16 as 8 per-head-pair pieces so attention on pair i starts
as soon as piece i has arrived; qT projections are interleaved with the k
pieces. Softmax = exp(sim)*exp(biasT) (no max subtraction; values are
small). The softmax denominator comes from 4-way column-tiled ones-weight
matmuls; attn@v packs the two heads of a pair onto PE column halves via
tile_position. PSUM accumulators are zero-initialized with start=True
zero-weight matmuls so interleaved accumulation groups sharing a bank never
clear each other's has_written bits.
"""

import numpy as np

CORES = 8
B = 4
N = 2048
NLOC = N // CORES          # 256
ROWS = B * NLOC            # 1024 local query rows (row = b*NLOC + q)
DIM = 1024
H = 16
D = 64
PAIRS = H // 2             # head pairs
KC = N // 128              # 16 kv chunks of 128 tokens per batch
KP = 128 * DIM             # k^T part of one AG piece (elements)
VP = ROWS * 128            # v part of one AG piece
PIECE = KP + VP            # per-rank payload of one piece (bf16 elements)
SCALE = D ** -0.5
EPS = 1e-5

_CACHE = {}


def _build_nc():
    import contextlib
    import concourse.bass as bass
    import concourse.bacc as bacc
    import concourse.tile as tile
    import concourse.mybir as mybir
    from concourse import masks

    f32 = mybir.dt.float32
    bf16 = mybir.dt.bfloat16
    AF = mybir.ActivationFunctionType
    ALU = mybir.AluOpType

    nc = bacc.Bacc("TRN2", target_bir_lowering=False, debug=False,
                   num_devices=CORES)

    x_in = nc.dram_tensor("x", [ROWS, DIM], f32, kind="ExternalInput")
    # pre-transposed kv-major bias, bf16: [h][p][c*256+q], kv = c*128+p
    bias_in = nc.dram_tensor("attn_bias", [H, 128, KC * NLOC], bf16,
                             kind="ExternalInput")
    ln_g = nc.dram_tensor("ln_g", [DIM], f32, kind="ExternalInput")
    ln_b = nc.dram_tensor("ln_b", [DIM], f32, kind="ExternalInput")
    wq_in = nc.dram_tensor("Wq", [DIM, DIM], bf16, kind="ExternalInput")
    wkv_in = nc.dram_tensor("Wkv", [DIM, 2 * DIM], bf16, kind="ExternalInput")
    wout_in = nc.dram_tensor("Wout", [DIM, DIM], bf16, kind="ExternalInput")
    out_ext = nc.dram_tensor("out", [ROWS, DIM], f32, kind="ExternalOutput")

    with tile.TileContext(nc) as tc, contextlib.ExitStack() as top:
        # ------------------------------------------------------------------
        # DRAM scratch
        dram = top.enter_context(tc.tile_pool(name="dram", bufs=1, space="DRAM"))
        kv_loc = [dram.tile([PIECE], bf16, name=f"kvl{i}") for i in range(PAIRS)]
        kv_ful = [dram.tile([CORES * PIECE], bf16, name=f"kvf{i}",
                            addr_space="Shared") for i in range(PAIRS)]

        # ------------------------------------------------------------------
        # Constants
        cpool = top.enter_context(tc.tile_pool(name="consts", bufs=1))
        identity = cpool.tile([128, 128], f32, name="identity")
        masks.make_identity(nc, identity[:])
        eps_t = cpool.tile([128, 1], f32, name="eps_t")
        nc.vector.memset(eps_t[:], EPS)
        # selector for denominator broadcast: sel[h, j] = 1 iff j//64 == h
        sel = cpool.tile([H, H * D], f32, name="sel")
        ones64 = cpool.tile([32, D], f32, name="ones64")
        nc.gpsimd.memset(sel[:], 0.0)
        nc.gpsimd.memset(ones64[:], 1.0)
        for h in range(H):
            nc.gpsimd.dma_start(sel[h:h + 1, h * D:(h + 1) * D], ones64[0:1, :])
        zeros128 = cpool.tile([128, 128], bf16, name="zeros128")
        nc.vector.memset(zeros128[:], 0.0)
        ones1 = cpool.tile([128, 1], bf16, name="ones1")
        nc.vector.memset(ones1[:], 1.0)
        denom_all = cpool.tile([H, ROWS], f32, name="denom_all")

        # ------------------------------------------------------------------
        # Persistent pools (live until the end; LIFO with the top stack)
        qt_pool = top.enter_context(tc.tile_pool(name="qT", bufs=1))
        asm_pool = top.enter_context(tc.tile_pool(name="asm", bufs=1))
        asm = [asm_pool.tile([128, ROWS], bf16, name=f"asm{i}") for i in range(8)]

        # ------------------------------------------------------------------
        # Weights: bf16 direct HWDGE loads (cast done host-side).
        w_stack = contextlib.ExitStack()
        wq_pool = w_stack.enter_context(tc.tile_pool(name="wq", bufs=1))
        wkv_pool = w_stack.enter_context(tc.tile_pool(name="wkv", bufs=1))
        wq_bf, wkv_bf = [], []
        for t in range(8):
            wkt = wkv_pool.tile([128, 2 * DIM], bf16, name=f"wkv{t}")
            nc.gpsimd.dma_start(wkt[:], wkv_in[t * 128:(t + 1) * 128, :])
            wkv_bf.append(wkt)
        for t in range(8):
            wqt = wq_pool.tile([128, DIM], bf16, name=f"wq{t}")
            nc.gpsimd.dma_start(wqt[:], wq_in[t * 128:(t + 1) * 128, :])
            wq_bf.append(wqt)

        # xnT pool opened before the LN pool so LN tiles can be freed first
        xnt_stack = contextlib.ExitStack()
        xnt_pool = xnt_stack.enter_context(tc.tile_pool(name="xnT", bufs=1))

        # ------------------------------------------------------------------
        # Phase 1: LayerNorm (rows on partitions) -> xn f32 in place
        ln_pool = contextlib.ExitStack()
        xpool = ln_pool.enter_context(tc.tile_pool(name="x", bufs=1))
        spool = ln_pool.enter_context(tc.tile_pool(name="stats", bufs=1))
        g_t = spool.tile([128, DIM], f32, name="g_t")
        b_t = spool.tile([128, DIM], f32, name="b_t")
        nc.gpsimd.dma_start(
            out=g_t[:],
            in_=bass.AP(tensor=ln_g.ap().tensor, offset=0, ap=[[0, 128], [1, DIM]]))
        nc.gpsimd.dma_start(
            out=b_t[:],
            in_=bass.AP(tensor=ln_b.ap().tensor, offset=0, ap=[[0, 128], [1, DIM]]))
        x_t = []
        for s in range(8):
            xt = xpool.tile([128, DIM], f32, name=f"x{s}")
            nc.sync.dma_start(xt[:], x_in[s * 128:(s + 1) * 128, :])
            stats = spool.tile([128, 2, 6], f32, name=f"st{s}")
            mv = spool.tile([128, 2], f32, name=f"mv{s}")
            for g in range(2):
                nc.vector.bn_stats(stats[:, g], xt[:, g * 512:(g + 1) * 512])
            nc.vector.bn_aggr(mv[:], stats[:])
            # rstd = 1/sqrt(var + eps)
            nc.scalar.activation(mv[:, 1:2], mv[:, 1:2], AF.Sqrt,
                                 bias=eps_t[:, 0:1], scale=1.0)
            nc.vector.reciprocal(mv[:, 1:2], mv[:, 1:2])
            nc.vector.tensor_scalar(out=xt[:], in0=xt[:],
                                    scalar1=mv[:, 0:1], scalar2=mv[:, 1:2],
                                    op0=ALU.subtract, op1=ALU.mult)
            nc.vector.tensor_mul(xt[:], xt[:], g_t[:])
            nc.vector.tensor_add(xt[:], xt[:], b_t[:])
            x_t.append(xt)

        # ------------------------------------------------------------------
        # Phase 2: transpose xn -> xnT bf16 [dim-part, row-free]
        tr_stack = contextlib.ExitStack()
        tr_pool = tr_stack.enter_context(
            tc.tile_pool(name="trps", bufs=2, space="PSUM"))
        xnT = []
        for t in range(8):
            ps = tr_pool.tile([128, ROWS], f32, name="trp")
            for s in range(8):
                nc.tensor.transpose(ps[:, s * 128:(s + 1) * 128],
                                    x_t[s][:, t * 128:(t + 1) * 128],
                                    identity[:])
            xt_b = xnt_pool.tile([128, ROWS], bf16, name=f"xnT{t}")
            nc.vector.tensor_copy(xt_b[:], ps[:])
            xnT.append(xt_b)
        tr_stack.close()
        ln_pool.close()

        # ------------------------------------------------------------------
        # Phase 4: QKV projections (bf16) + per-piece kv bounce + AllGathers
        qkv_psum_stack = contextlib.ExitStack()
        qkv_psum = qkv_psum_stack.enter_context(
            tc.tile_pool(name="qkvp", bufs=2, space="PSUM"))
        stage_stack = contextlib.ExitStack()
        stage_pool = stage_stack.enter_context(tc.tile_pool(name="kvstage", bufs=4))

        # v first (every piece needs all of v), then per pair: k piece +
        # AllGather + q projection (so pair 0 can start attention asap).
        qT = []
        for s in range(8):
            ps = qkv_psum.tile([128, DIM], f32, name="qkvps")
            for ki in range(8):
                for nh in range(2):
                    nc.tensor.matmul(ps[:, nh * 512:(nh + 1) * 512],
                                     xnT[ki][:, s * 128:(s + 1) * 128],
                                     wkv_bf[ki][:, DIM + nh * 512:DIM + (nh + 1) * 512],
                                     start=(ki == 0), stop=(ki == 7))
            vst = stage_pool.tile([128, DIM], bf16, name="kvst")
            nc.vector.tensor_copy(vst[:], ps[:])
            for i in range(PAIRS):
                kvl = kv_loc[i][:]
                nc.sync.dma_start(
                    out=bass.AP(tensor=kvl.tensor,
                                offset=kvl.offset + KP + s * 128 * 128,
                                ap=[[128, 128], [1, 128]]),
                    in_=vst[:, i * 128:(i + 1) * 128])

        for i in range(PAIRS):
            ps = qkv_psum.tile([128, ROWS], f32, name="qkvps")
            for ki in range(8):
                for nh in range(2):
                    nc.tensor.matmul(ps[:, nh * 512:(nh + 1) * 512],
                                     wkv_bf[ki][:, i * 128:(i + 1) * 128],
                                     xnT[ki][:, nh * 512:(nh + 1) * 512],
                                     start=(ki == 0), stop=(ki == 7))
            kst = stage_pool.tile([128, ROWS], bf16, name="kvst")
            nc.vector.tensor_copy(kst[:], ps[:])
            kvl = kv_loc[i][:]
            nc.sync.dma_start(
                out=bass.AP(tensor=kvl.tensor, offset=kvl.offset,
                            ap=[[DIM, 128], [1, DIM]]),
                in_=kst[:])
            nc.gpsimd.collective_compute(
                "AllGather",
                mybir.AluOpType.bypass,
                replica_groups=[list(range(CORES))],
                ins=[kv_loc[i][:].opt()],
                outs=[kv_ful[i][:].opt()],
            )
            # interleave q projection for this pair
            ps = qkv_psum.tile([128, ROWS], f32, name="qkvps")
            for ki in range(8):
                for nh in range(2):
                    nc.tensor.matmul(ps[:, nh * 512:(nh + 1) * 512],
                                     wq_bf[ki][:, i * 128:(i + 1) * 128],
                                     xnT[ki][:, nh * 512:(nh + 1) * 512],
                                     start=(ki == 0), stop=(ki == 7))
            qtile = qt_pool.tile([128, ROWS], bf16, name=f"qT{i}")
            nc.vector.tensor_scalar_mul(qtile[:], ps[:], SCALE)
            qT.append(qtile)

        stage_stack.close()
        qkv_psum_stack.close()
        xnt_stack.close()
        w_stack.close()

        # ------------------------------------------------------------------
        # Phase 5: attention over head pairs (software-pipelined over chunks)

        attn_stack = contextlib.ExitStack()
        kt_pool = attn_stack.enter_context(tc.tile_pool(name="kT", bufs=3))
        vt_pool = attn_stack.enter_context(tc.tile_pool(name="vt", bufs=2))
        ebt_pool = attn_stack.enter_context(tc.tile_pool(name="ebT", bufs=4))
        ae_pool = attn_stack.enter_context(tc.tile_pool(name="ae", bufs=6))
        den_pool = attn_stack.enter_context(tc.tile_pool(name="den", bufs=2))
        sim_psum = attn_stack.enter_context(
            tc.tile_pool(name="simp", bufs=2, space="PSUM"))
        out_psum = attn_stack.enter_context(
            tc.tile_pool(name="outp", bufs=2, space="PSUM"))
        dn_psum = attn_stack.enter_context(
            tc.tile_pool(name="dnp", bufs=2, space="PSUM"))

        # biasT tiles: [128 p, KC*256] bf16 per head; load + exp in-place.
        ebt_tiles = {}

        def load_ebt(h):
            et = ebt_pool.tile([128, KC * NLOC], bf16, name="ebt")
            nc.scalar.dma_start(et[:], bias_in[h])
            nc.scalar.activation(et[:], et[:], AF.Exp)
            ebt_tiles[h] = et

        load_ebt(0)
        load_ebt(1)

        for i in range(PAIRS):
            kvf = kv_ful[i][:]
            KVF_T = kvf.tensor
            # k^T for the head pair: [128 (2 heads x 64 d), B*N] bf16
            kt = kt_pool.tile([128, B * N], bf16, name="kt")
            ktd = kt[:]
            for b in range(B):
                nc.sync.dma_start(
                    out=bass.AP(tensor=ktd.tensor, offset=ktd.offset + b * N,
                                ap=[ktd.ap[0], [NLOC, CORES], [1, NLOC]]),
                    in_=bass.AP(tensor=KVF_T,
                                offset=kvf.offset + b * NLOC,
                                ap=[[DIM, 128], [PIECE, CORES], [1, NLOC]]))
            # v for both heads of the pair: [128 tok, (b,c) x 128 inner]
            vt = vt_pool.tile([128, B * KC * 128], bf16, name="vt")
            vb = vt[:]
            for b in range(B):
                for c2 in range(2):
                    nc.sync.dma_start(
                        out=bass.AP(tensor=vb.tensor,
                                    offset=(vb.offset + (b * KC + c2) * 128),
                                    ap=[vb.ap[0], [256, CORES], [1, 128]]),
                        in_=bass.AP(tensor=KVF_T,
                                    offset=(kvf.offset + KP
                                            + (b * NLOC + c2 * 128) * 128),
                                    ap=[[128, 128], [PIECE, CORES], [1, 128]]))
            # prefetch next pair's exp(bias^T)
            if i + 1 < PAIRS:
                load_ebt(2 * (i + 1))
                load_ebt(2 * (i + 1) + 1)
            ebt = [ebt_tiles.pop(2 * i), ebt_tiles.pop(2 * i + 1)]

            po, dn = {}, {}
            for bp in range(2):
                p_t = out_psum.tile([128, 512], f32, name="po")
                po[bp] = p_t
                nc.tensor.matmul(p_t[:, :], zeros128[:], qT[i][:, 0:512],
                                 start=True, stop=False, skip_group_check=True)
            d_t = dn_psum.tile([97, 512], f32, name="dn")
            for cg in range(4):
                nc.tensor.matmul(d_t[32 * cg:32 * cg + 1, :],
                                 zeros128[:, 0:1], qT[i][:, 0:512],
                                 start=True, stop=False, skip_group_check=True,
                                 tile_position=(0, 32 * cg))

            ae_ring = {}
            for c in range(KC + 1):
                if c < KC:
                    pss = {}
                    for parity in range(2):
                        pss[parity] = sim_psum.tile([128, B * NLOC], f32,
                                                    name="simps")
                    # interleave parities so consecutive LDWEIGHTS hit
                    # different PE row-groups and overlap the matmuls
                    for b in range(B):
                        for parity in range(2):
                            nc.tensor.matmul(
                                pss[parity][:, b * NLOC:(b + 1) * NLOC],
                                kt[parity * 64:parity * 64 + 64,
                                   b * N + c * 128:b * N + (c + 1) * 128],
                                qT[i][parity * 64:parity * 64 + 64,
                                      b * NLOC:(b + 1) * NLOC],
                                start=True, stop=True,
                                tile_position=(parity * 64, 0))
                    for parity in range(2):
                        ae = ae_pool.tile([128, B * NLOC], bf16, name="ae")
                        nc.scalar.activation(ae[:], pss[parity][:], AF.Exp)
                        ebs = ebt[parity][:, c * NLOC:(c + 1) * NLOC]
                        bcast = bass.AP(tensor=ebs.tensor, offset=ebs.offset,
                                        ap=[ebs.ap[0], [0, B], [1, NLOC]])
                        ae3 = ae[:].rearrange("p (b q) -> p b q", b=B)
                        nc.vector.tensor_tensor(out=ae3, in0=ae3, in1=bcast,
                                                op=ALU.mult)
                        ae_ring[c, parity] = ae
                if c >= 1:
                    cp = c - 1
                    ae_e = ae_ring.pop((cp, 0))
                    ae_o = ae_ring.pop((cp, 1))
                    for b in range(B):
                        blk = (b * KC + cp) * 128
                        # attn@v: head-even -> PE cols 0-63, head-odd ->
                        # cols 64-127; the two matmuls co-execute
                        nc.tensor.matmul(
                            po[b // 2][0:64, (b % 2) * NLOC:((b % 2) + 1) * NLOC],
                            vt[:, blk:blk + 64],
                            ae_e[:, b * NLOC:(b + 1) * NLOC],
                            start=False, stop=(cp == KC - 1),
                            tile_position=(0, 0), skip_group_check=True)
                        nc.tensor.matmul(
                            po[b // 2][64:128, (b % 2) * NLOC:((b % 2) + 1) * NLOC],
                            vt[:, blk + 64:blk + 128],
                            ae_o[:, b * NLOC:(b + 1) * NLOC],
                            start=False, stop=(cp == KC - 1),
                            tile_position=(0, 64), skip_group_check=True)
                    # denominators: 4-way column-tiled ones-weight matmuls
                    for cg, (ae_t, bp) in enumerate(
                            [(ae_e, 0), (ae_o, 0), (ae_e, 1), (ae_o, 1)]):
                        nc.tensor.matmul(
                            d_t[32 * cg:32 * cg + 1, :], ones1[:],
                            ae_t[:, bp * 512:(bp + 1) * 512],
                            start=False, stop=(cp == KC - 1),
                            tile_position=(0, 32 * cg), skip_group_check=True)

            # evacuate pair outputs + denominators
            den_row = den_pool.tile([97, ROWS], f32, name="den")
            for bp in range(2):
                nc.vector.tensor_copy(
                    asm[i][:, bp * 512:(bp + 1) * 512], po[bp][:, :])
            for cg, (par, bp) in enumerate([(0, 0), (1, 0), (0, 1), (1, 1)]):
                nc.vector.tensor_copy(
                    den_row[32 * par:32 * par + 1, bp * 512:(bp + 1) * 512],
                    d_t[32 * cg:32 * cg + 1, :])
            # SBUF->SBUF partition-shift into denom_all rows 2i, 2i+1
            nc.gpsimd.dma_start(denom_all[2 * i:2 * i + 1, :], den_row[0:1, :])
            nc.gpsimd.dma_start(denom_all[2 * i + 1:2 * i + 2, :],
                                den_row[32:33, :])

        attn_stack.close()

        # ------------------------------------------------------------------
        # Phase 6: normalize by softmax denominator
        fin_stack = contextlib.ExitStack()
        rb_psum = fin_stack.enter_context(
            tc.tile_pool(name="rbp", bufs=2, space="PSUM"))
        rb_pool = fin_stack.enter_context(tc.tile_pool(name="rbs", bufs=2))
        recip = cpool.tile([H, ROWS], f32, name="recip")
        nc.vector.reciprocal(recip[:], denom_all[:])
        for i in range(8):
            ps = rb_psum.tile([128, ROWS], f32, name="rbp")
            for nh in range(2):
                nc.tensor.matmul(ps[:, nh * 512:(nh + 1) * 512],
                                 sel[:, i * 128:(i + 1) * 128],
                                 recip[:, nh * 512:(nh + 1) * 512],
                                 start=True, stop=True)
            rbs = rb_pool.tile([128, ROWS], f32, name="rbs")
            nc.vector.tensor_copy(rbs[:], ps[:])
            nc.vector.tensor_mul(asm[i][:], asm[i][:], rbs[:])

        # ------------------------------------------------------------------
        # Phase 7: output projection
        wo_pool = fin_stack.enter_context(tc.tile_pool(name="wo", bufs=1))
        wout_bf = []
        for t in range(8):
            wot = wo_pool.tile([128, DIM], bf16, name=f"wo{t}")
            nc.sync.dma_start(wot[:], wout_in[t * 128:(t + 1) * 128, :])
            wout_bf.append(wot)
        f_psum = fin_stack.enter_context(
            tc.tile_pool(name="fp", bufs=2, space="PSUM"))
        o_pool = fin_stack.enter_context(tc.tile_pool(name="osb", bufs=3))
        for mi in range(8):
            ps = f_psum.tile([128, DIM], f32, name="fp")
            for ki in range(8):
                for nh in range(2):
                    nc.tensor.matmul(ps[:, nh * 512:(nh + 1) * 512],
                                     asm[ki][:, mi * 128:(mi + 1) * 128],
                                     wout_bf[ki][:, nh * 512:(nh + 1) * 512],
                                     start=(ki == 0), stop=(ki == 7))
            ot = o_pool.tile([128, DIM], f32, name="ot")
            nc.vector.tensor_copy(ot[:], ps[:])
            nc.sync.dma_start(out_ext[mi * 128:(mi + 1) * 128, :], ot[:])

        fin_stack.close()

    nc.finalize()
    return nc


def _get_nc():
    if "nc" not in _CACHE:
        _CACHE["nc"] = _build_nc()
    return _CACHE["nc"]


def prep_in_maps(x, attn_bias, ln_g, ln_b, Wq, Wkv, Wout):
    """Host-side sharding + layout prep (slice/transpose/cast only)."""
    import ml_dtypes

    bf16 = ml_dtypes.bfloat16
    x = np.asarray(x, dtype=np.float32)
    attn_bias = np.asarray(attn_bias, dtype=np.float32)
    wq = np.asarray(Wq, dtype=np.float32).astype(bf16)
    wkv = np.asarray(Wkv, dtype=np.float32).astype(bf16)
    wout = np.asarray(Wout, dtype=np.float32).astype(bf16)
    ln_g = np.asarray(ln_g, dtype=np.float32)
    ln_b = np.asarray(ln_b, dtype=np.float32)
    in_maps = []
    for r in range(CORES):
        # bias slice [H, 256 q, 2048 kv] -> kv-major [H, 128 p, KC, 256 q]
        bslice = attn_bias[:, r * NLOC:(r + 1) * NLOC, :]
        bT = np.ascontiguousarray(
            bslice.reshape(H, NLOC, KC, 128).transpose(0, 3, 2, 1)
        ).astype(bf16).reshape(H, 128, KC * NLOC)
        in_maps.append({
            "x": np.ascontiguousarray(
                x[:, r * NLOC:(r + 1) * NLOC, :]).reshape(ROWS, DIM),
            "attn_bias": bT,
            "ln_g": ln_g,
            "ln_b": ln_b,
            "Wq": wq,
            "Wkv": wkv,
            "Wout": wout,
        })
    return in_maps


def kernel(x, attn_bias, ln_g, ln_b, Wq, Wkv, Wout):
    from concourse import bass_utils

    nc = _get_nc()
    in_maps = prep_in_maps(x, attn_bias, ln_g, ln_b, Wq, Wkv, Wout)
    res = bass_utils.run_bass_kernel_spmd(nc, in_maps, core_ids=list(range(CORES)))
    out = np.empty((B, N, DIM), dtype=np.float32)
    for r in range(CORES):
        out[:, r * NLOC:(r + 1) * NLOC, :] = \
            res.results[r]["out"].reshape(B, NLOC, DIM)
    return out
